# revision 5
# baseline (speedup 1.0000x reference)
"""ContrastLoss (InfoNCE-style) Trainium2 kernel — PE-pairwise edition.

Math (per sample b):
    s[i,j] = (tmap[b,i,j] . qhat[b]) / ||tmap[b,i,j]||      (qhat = normalized pos_query)
    e = exp(s); num = sum(e * pos_mask); den = num + sum(e * neg_mask)
    li = -log(num / (den + EPS)); loss = mean(li over valid samples)

Only ~35% of cells are masked (pos|neg); the rest contribute nothing.
Per core (4 samples):
  - Host: normalize q, cast the tmap shard to fp16 as a (16384+4, 256) row
    table whose last rows are the qhat vectors. Masked cells are packed into
    T tiles of 128 slots (127 cells of one sample + that sample's qhat in
    slot 127), with per-slot pos/neg weights (pad: row 0, weight 0).
  - Device: index DMA first, then G transpose-mode dma_gathers (fp16,
    H-on-partitions; descriptor-gens up-front on Pool). Per tile the
    TensorEngine computes the 128x128 pairwise Gram matrix in PSUM over the
    two H-chunks; its diagonal is ||t||^2 per slot and column 127 is
    dot(t, qhat). DVE extracts the diagonal (one 128-elem multiply-reduce
    with an identity), ScalarE copies the dot column. Epilogue on (128, T)
    stat tiles: rsqrt via ln/exp, exp, per-sample mask-weighted reduces ->
    (128, 2*BS) partials.
  - Host: partial sums -> per-sample num/den, -log, valid masking, mean.
"""

import numpy as np

import concourse.bacc as bacc
import concourse.tile as tile
from concourse import mybir
from concourse.bass_utils import run_bass_kernel_spmd
from concourse.hw_specs import get_activation_tables as _real_gat

_ACT_SET = "natural_log_exp_and_others"  # contains ln, exp


def _patched_gat(arch):
    """Force every activation to resolve to the one set containing all our
    functions (ln/exp), avoiding per-use table-set thrashing."""
    tabs = _real_gat(arch)
    return {k: (v if k == _ACT_SET else set()) for k, v in tabs.items()}


bacc.get_activation_tables = _patched_gat

N_CORES = 8
B, S, H = 32, 64, 256
BS = B // N_CORES           # samples per core
CELLS = S * S               # 4096 cells per sample
ROWS = BS * CELLS           # 16384 table rows per core
TROWS = ROWS + BS           # + qhat rows
CPT = 127                   # cells per tile (slot 127 = qhat)
EPS = 1e-8

N_PSUM = 8                  # rotating PSUM pairwise tiles
# stage toggles for differential hardware timing (leave True for real runs)
DO_MM = True
DO_DIAG = True
DO_DOT = True
DO_EPI = True
N_SCR = 4                   # rotating DVE diag scratch tiles

# chunk schedule in tiles; -1 entries split the remainder evenly
CHUNK_PLAN = [2, 4, 7, 7, 7, 7, 7, 7]

_NC_CACHE = {}


def _chunks(T):
    plan = list(CHUNK_PLAN)
    fixed = sum(x for x in plan if x > 0)
    nfree = sum(1 for x in plan if x < 0)
    rem = T - fixed
    if nfree == 0:
        out = plan[:-1] + [plan[-1] + rem]
    else:
        base, extra = divmod(rem, nfree)
        out = []
        i = 0
        for x in plan:
            if x > 0:
                out.append(x)
            else:
                out.append(base + (1 if i < extra else 0))
                i += 1
    out = [c for c in out if c > 0]
    assert sum(out) == T and all(c > 0 for c in out)
    return out


def _build_nc(T, tile_bounds, loop_reps=0):
    """T: total tiles. tile_bounds: per-sample (start, end) tile ranges."""
    A = mybir.ActivationFunctionType
    OP = mybir.AluOpType
    dt = mybir.dt

    nc = bacc.Bacc(
        "TRN2",
        target_bir_lowering=False,
        debug=False,
        enable_asserts=False,
        num_devices=N_CORES,
        num_swdge_queues=4,
    )

    t_in = nc.dram_tensor("t_in", [TROWS, H], dt.float16, kind="ExternalInput").ap()
    w_in = nc.dram_tensor("w_in", [128, 2 * T], dt.float32, kind="ExternalInput").ap()
    ix_in = nc.dram_tensor("ix_in", [128, 8 * T], dt.int16, kind="ExternalInput").ap()
    id_in = nc.dram_tensor("id_in", [128, 128], dt.float32, kind="ExternalInput").ap()
    parts = nc.dram_tensor("parts", [128, 4 * BS], dt.float32, kind="ExternalOutput").ap()

    chunk_t = _chunks(T)
    G = len(chunk_t)

    with tile.TileContext(nc) as tc:
        with (
            tc.tile_pool(name="gat", bufs=1) as gpool,
            tc.tile_pool(name="small", bufs=1) as spool,
            tc.psum_pool(name="ps", bufs=N_PSUM) as ppool,
        ):
            # index DMA first: it gates the gather pipeline
            ixsb = spool.tile([128, 8 * T], dt.int16, tag="ixsb")
            nc.sync.dma_start(out=ixsb[:], in_=ix_in[:])
            idsb = spool.tile([128, 128], dt.float32, tag="idsb")
            nc.sync.dma_start(out=idsb[:], in_=id_in[:])
            wsb = spool.tile([128, 2 * T], dt.float32, tag="wsb")
            nc.sync.dma_start(out=wsb[:], in_=w_in[:])

            npart = spool.tile([128, 4 * BS], dt.float32, tag="npart")
            dscr = [spool.tile([128, 128], dt.float32, tag=f"dscr{i}",
                               name=f"dscr{i}") for i in range(N_SCR)]
            eps_scr = spool.tile([128, T], dt.float32, tag="eps_scr")
            dotb = spool.tile([128, T], dt.float32, tag="dotb")
            ssqb = spool.tile([128, T], dt.float32, tag="ssqb")

            import contextlib
            loop_cm = tc.For_i(0, loop_reps, 1) if loop_reps else contextlib.nullcontext()
            with loop_cm:
                nc.gpsimd.memset(npart[:], 0.0)
                # all gather descriptor-gens up-front on Pool
                gts = []
                t0 = 0
                for g in range(G):
                    tg = chunk_t[g]
                    gt = gpool.tile([128, 2, 128 * tg], dt.float16, tag=f"gt{g}",
                                    name=f"gt{g}")
                    nc.gpsimd.dma_gather(
                        out_ap=gt[:],
                        in_ap=t_in[:],
                        idxs_ap=ixsb[:, 8 * t0:8 * (t0 + tg)],
                        num_idxs=128 * tg,
                        num_idxs_reg=128 * tg,
                        elem_size=H,
                        transpose=True,
                        queue_num=g % 4,
                    )
                    gts.append((gt, t0, tg))
                    t0 += tg

                # Two-phase epilogue: phase 0 (all tiles except the last
                # chunk's) is emitted before the last chunk's per-tile ops, so
                # ACT/DVE chew it while waiting on the final gather; phase 1
                # covers the rest. Per-sample masked sums land in disjoint
                # npart column halves; host sums both.
                lnb = spool.tile([128, T], dt.float32, tag="lnb")
                invn = spool.tile([128, T], dt.float32, tag="invn")
                sb = spool.tile([128, T], dt.float32, tag="sb")
                eb = spool.tile([128, T], dt.float32, tag="eb")

                def epilogue(ph, pa, pb):
                    nc.scalar.activation(lnb[:, pa:pb], ssqb[:, pa:pb], A.Ln)
                    nc.scalar.activation(invn[:, pa:pb], lnb[:, pa:pb], A.Exp,
                                         scale=-0.5)
                    nc.vector.tensor_mul(sb[:, pa:pb], dotb[:, pa:pb],
                                         invn[:, pa:pb])
                    nc.scalar.activation(eb[:, pa:pb], sb[:, pa:pb], A.Exp)
                    for s in range(BS):
                        ta, tb = tile_bounds[s]
                        ta, tb = max(ta, pa), min(tb, pb)
                        if ta >= tb:
                            continue
                        col = 2 * s + (2 * BS) * ph
                        nc.vector.scalar_tensor_tensor(
                            out=eps_scr[:, ta:tb], in0=eb[:, ta:tb], scalar=0.0,
                            in1=wsb[:, ta:tb],
                            op0=OP.bypass, op1=OP.mult,
                            accum_out=npart[:, col:col + 1],
                        )
                        nc.vector.scalar_tensor_tensor(
                            out=eps_scr[:, ta:tb], in0=eb[:, ta:tb], scalar=0.0,
                            in1=wsb[:, T + ta:T + tb],
                            op0=OP.bypass, op1=OP.mult,
                            accum_out=npart[:, col + 1:col + 2],
                        )

                ph_split = T - chunk_t[-1]
                ntau = 0
                for g in range(G):
                    gt, t0, tg = gts[g]
                    for t in range(tg):
                        tau = t0 + t
                        sl = slice(128 * t, 128 * (t + 1))
                        if DO_MM:
                            P = ppool.tile([128, 128], dt.float32, tag="P",
                                           name="P")
                            nc.tensor.matmul(out=P[:], lhsT=gt[:, 0, sl],
                                             rhs=gt[:, 0, sl], start=True, stop=False)
                            nc.tensor.matmul(out=P[:], lhsT=gt[:, 1, sl],
                                             rhs=gt[:, 1, sl], start=False, stop=True)
                            if DO_DIAG:
                                nc.vector.scalar_tensor_tensor(
                                    out=dscr[ntau % N_SCR][:], in0=P[:], scalar=0.0,
                                    in1=idsb[:], op0=OP.bypass, op1=OP.mult,
                                    accum_out=ssqb[:, tau:tau + 1],
                                )
                            if DO_DOT:
                                nc.scalar.activation(
                                    dotb[:, tau:tau + 1], P[:, 127:128], A.Copy)
                        ntau += 1
                    if DO_EPI and g == G - 2 and ph_split > 0:
                        epilogue(0, 0, ph_split)
                if DO_EPI and ph_split < T:
                    epilogue(1, ph_split, T)

            nc.sync.dma_start(out=parts[:], in_=npart[:])

    nc.compile()
    return nc


def get_nc(loop_reps=0, T=None, tile_bounds=None):
    if T is None:
        T, tile_bounds = _NC_CACHE["last_layout"]
    key = (T, tuple(tile_bounds), loop_reps, DO_MM, DO_DIAG, DO_DOT, DO_EPI)
    if key not in _NC_CACHE:
        _NC_CACHE[key] = _build_nc(T, tile_bounds, loop_reps)
    return _NC_CACHE[key]


def _layout(mp, mn):
    """Per-core tile layout from the full masks. Returns (T, tile_bounds):
    tile_bounds[s] = (start, end) tile range of sample s — identical across
    cores (T_s = max over cores)."""
    tcounts = []
    for s in range(BS):
        mx = 0
        for c in range(N_CORES):
            n = int((mp[c * BS + s] | mn[c * BS + s]).sum())
            mx = max(mx, n)
        tcounts.append(max(1, -(-mx // CPT)))
    bounds = []
    t0 = 0
    for s in range(BS):
        bounds.append((t0, t0 + tcounts[s]))
        t0 += tcounts[s]
    return t0, tuple(bounds)


def _core_inputs(tm16, qhat16, mp, mn, T, tile_bounds):
    """Build one core's in_map. tm16: (ROWS, H) fp16; qhat16: (BS, H) fp16."""
    tab = np.zeros((TROWS, H), np.float16)
    tab[:ROWS] = tm16
    tab[ROWS:] = qhat16

    flat_idx = np.zeros(128 * T, np.int16)
    posw = np.zeros((128, T), np.float32)
    negw = np.zeros((128, T), np.float32)
    for s in range(BS):
        ta, tb = tile_bounds[s]
        # q slot (row 127) of every tile of this sample
        for tau in range(ta, tb):
            flat_idx[tau * 128 + 127] = ROWS + s
        m = (mp[s] | mn[s]).reshape(-1)
        cells = np.nonzero(m)[0]
        k = cells.size
        assert k <= (tb - ta) * CPT
        isp = mp[s].reshape(-1)[cells]
        kk = np.arange(k)
        tau = ta + kk // CPT
        p = kk % CPT
        flat_idx[tau * 128 + p] = (s * CELLS + cells).astype(np.int16)
        posw[p, tau] = isp.astype(np.float32)
        negw[p, tau] = (~isp).astype(np.float32)
    ix = flat_idx.reshape(8 * T, 16).T  # (16, 8T): slot g -> [g%16, g//16]
    ix = np.tile(ix, (8, 1)).copy()     # (128, 8T)
    return {
        "t_in": tab,
        "w_in": np.ascontiguousarray(np.concatenate([posw, negw], axis=1)),
        "ix_in": np.ascontiguousarray(ix),
        "id_in": np.eye(128, dtype=np.float32),
    }


def make_in_maps(pos_query, tmap, mask2d_pos, mask2d_neg):
    pq = np.asarray(pos_query, dtype=np.float32)
    tm = np.asarray(tmap, dtype=np.float32)
    mp = np.asarray(mask2d_pos).astype(bool)
    mn = np.asarray(mask2d_neg).astype(bool)

    T, tile_bounds = _layout(mp, mn)

    qn = np.sqrt(np.sum(pq * pq, axis=-1, keepdims=True, dtype=np.float32))
    qhat16 = (pq / (qn + np.float32(EPS))).astype(np.float16)
    tm16 = tm.astype(np.float16).reshape(N_CORES, ROWS, H)

    in_maps = []
    for c in range(N_CORES):
        sl = slice(c * BS, (c + 1) * BS)
        in_maps.append(_core_inputs(
            np.ascontiguousarray(tm16[c]), qhat16[sl], mp[sl], mn[sl],
            T, tile_bounds))
    return in_maps, mp, mn, T, tile_bounds


def finish(parts_per_core, mp, mn):
    """parts_per_core: list of (128, 4*BS) arrays (two phase-halves of
    per-sample partials) -> scalar loss."""
    num = np.zeros(B, np.float32)
    neg = np.zeros(B, np.float32)
    for c in range(N_CORES):
        p = parts_per_core[c]
        for s in range(BS):
            num[c * BS + s] = (p[:, 2 * s].sum(dtype=np.float32)
                               + p[:, 2 * BS + 2 * s].sum(dtype=np.float32))
            neg[c * BS + s] = (p[:, 2 * s + 1].sum(dtype=np.float32)
                               + p[:, 2 * BS + 2 * s + 1].sum(dtype=np.float32))
    den = num + neg
    with np.errstate(divide="ignore", invalid="ignore", over="ignore"):
        li = -np.log(num / (den + np.float32(EPS)))
    valid = mp.any(axis=(1, 2)) & mn.any(axis=(1, 2))
    n_valid = max(int(valid.sum()), 1)
    loss = np.where(valid, li, np.float32(0.0)).sum(dtype=np.float32) / np.float32(n_valid)
    return np.asarray(loss, dtype=np.float32)


def kernel(pos_query, tmap, mask2d_pos, mask2d_neg):
    in_maps, mp, mn, T, tile_bounds = make_in_maps(
        pos_query, tmap, mask2d_pos, mask2d_neg)
    _NC_CACHE["last_layout"] = (T, tile_bounds)
    nc = get_nc(T=T, tile_bounds=tile_bounds)
    res = run_bass_kernel_spmd(nc, in_maps, list(range(N_CORES)))
    parts_per_core = [res.results[c]["parts"] for c in range(N_CORES)]
    return finish(parts_per_core, mp, mn)


if __name__ == "__main__":
    rng = np.random.default_rng(0)
    inputs = {
        "pos_query": rng.standard_normal((B, H), dtype=np.float32),
        "tmap": rng.standard_normal((B, S, S, H), dtype=np.float32),
        "mask2d_pos": rng.random((B, S, S)) < 0.05,
        "mask2d_neg": (rng.random((B, S, S)) >= 0.05) & (rng.random((B, S, S)) < 0.35),
    }
    print(kernel(**inputs))


# revision 6
# speedup vs baseline: 1.0901x; 1.0901x over previous
"""ContrastLoss (InfoNCE-style) Trainium2 kernel — PE-pairwise edition.

Math (per sample b):
    s[i,j] = (tmap[b,i,j] . qhat[b]) / ||tmap[b,i,j]||      (qhat = normalized pos_query)
    e = exp(s); num = sum(e * pos_mask); den = num + sum(e * neg_mask)
    li = -log(num / (den + EPS)); loss = mean(li over valid samples)

Only ~35% of cells are masked (pos|neg); the rest contribute nothing.
Per core (4 samples):
  - Host: normalize q, cast the tmap shard to fp16 as a (16384+4, 256) row
    table whose last rows are the qhat vectors. Masked cells are packed into
    T tiles of 128 slots (127 cells of one sample + that sample's qhat in
    slot 127), with per-slot pos/neg weights (pad: row 0, weight 0).
  - Device: index DMA first, then G transpose-mode dma_gathers (fp16,
    H-on-partitions; descriptor-gens up-front on Pool). Per tile the
    TensorEngine computes the 128x128 pairwise Gram matrix in PSUM over the
    two H-chunks; its diagonal is ||t||^2 per slot and column 127 is
    dot(t, qhat). DVE extracts the diagonal (one 128-elem multiply-reduce
    with an identity), ScalarE copies the dot column. Epilogue on (128, T)
    stat tiles: rsqrt via ln/exp, exp, per-sample mask-weighted reduces ->
    (128, 2*BS) partials.
  - Host: partial sums -> per-sample num/den, -log, valid masking, mean.
"""

import numpy as np

import concourse.bacc as bacc
import concourse.tile as tile
from concourse import mybir
from concourse.bass_utils import run_bass_kernel_spmd
from concourse.hw_specs import get_activation_tables as _real_gat

_ACT_SET = "natural_log_exp_and_others"  # contains ln, exp


def _patched_gat(arch):
    """Force every activation to resolve to the one set containing all our
    functions (ln/exp), avoiding per-use table-set thrashing."""
    tabs = _real_gat(arch)
    return {k: (v if k == _ACT_SET else set()) for k, v in tabs.items()}


bacc.get_activation_tables = _patched_gat

N_CORES = 8
B, S, H = 32, 64, 256
BS = B // N_CORES           # samples per core
CELLS = S * S               # 4096 cells per sample
ROWS = BS * CELLS           # 16384 table rows per core
TROWS = ROWS + BS           # + qhat rows
CPT = 127                   # cells per tile (slot 127 = qhat)
EPS = 1e-8

N_PSUM = 6                  # rotating PSUM pairwise tiles
# stage toggles for differential hardware timing (leave True for real runs)
DO_MM = True
DO_DIAG = True
DO_DOT = True
DO_EPI = True
N_SCR = 4                   # rotating DVE diag scratch tiles

# chunk schedule in tiles; -1 entries split the remainder evenly
CHUNK_PLAN = [3, 5, 7, 7, 7, 7, 7, 5]

_NC_CACHE = {}


def _chunks(T):
    plan = list(CHUNK_PLAN)
    fixed = sum(x for x in plan if x > 0)
    nfree = sum(1 for x in plan if x < 0)
    rem = T - fixed
    if nfree == 0:
        out = plan[:-1] + [plan[-1] + rem]
    else:
        base, extra = divmod(rem, nfree)
        out = []
        i = 0
        for x in plan:
            if x > 0:
                out.append(x)
            else:
                out.append(base + (1 if i < extra else 0))
                i += 1
    out = [c for c in out if c > 0]
    assert sum(out) == T and all(c > 0 for c in out)
    return out


def _build_nc(T, tile_bounds, loop_reps=0):
    """T: total tiles. tile_bounds: per-sample (start, end) tile ranges."""
    A = mybir.ActivationFunctionType
    OP = mybir.AluOpType
    dt = mybir.dt

    nc = bacc.Bacc(
        "TRN2",
        target_bir_lowering=False,
        debug=False,
        enable_asserts=False,
        num_devices=N_CORES,
        num_swdge_queues=4,
    )

    t_in = nc.dram_tensor("t_in", [TROWS, H], dt.float16, kind="ExternalInput").ap()
    w_in = nc.dram_tensor("w_in", [128, 2 * T], dt.float32, kind="ExternalInput").ap()
    ix_in = nc.dram_tensor("ix_in", [128, 8 * T], dt.int16, kind="ExternalInput").ap()
    id_in = nc.dram_tensor("id_in", [128, 128], dt.float32, kind="ExternalInput").ap()
    parts = nc.dram_tensor("parts", [128, 4 * BS], dt.float32, kind="ExternalOutput").ap()

    chunk_t = _chunks(T)
    G = len(chunk_t)

    with tile.TileContext(nc) as tc:
        with (
            tc.tile_pool(name="gat", bufs=1) as gpool,
            tc.tile_pool(name="small", bufs=1) as spool,
            tc.psum_pool(name="ps", bufs=N_PSUM) as ppool,
        ):
            # index DMA first: it gates the gather pipeline
            ixsb = spool.tile([128, 8 * T], dt.int16, tag="ixsb")
            nc.sync.dma_start(out=ixsb[:], in_=ix_in[:])
            idsb = spool.tile([128, 128], dt.float32, tag="idsb")
            nc.sync.dma_start(out=idsb[:], in_=id_in[:])
            wsb = spool.tile([128, 2 * T], dt.float32, tag="wsb")
            nc.sync.dma_start(out=wsb[:], in_=w_in[:])

            npart = spool.tile([128, 4 * BS], dt.float32, tag="npart")
            dscr = [spool.tile([128, 128], dt.float32, tag=f"dscr{i}",
                               name=f"dscr{i}") for i in range(N_SCR)]
            eps_scr = spool.tile([128, T], dt.float32, tag="eps_scr")
            dotb = spool.tile([128, T], dt.float32, tag="dotb")
            ssqb = spool.tile([128, T], dt.float32, tag="ssqb")

            import contextlib
            loop_cm = tc.For_i(0, loop_reps, 1) if loop_reps else contextlib.nullcontext()
            with loop_cm:
                nc.gpsimd.memset(npart[:], 0.0)
                # all gather descriptor-gens up-front on Pool
                gts = []
                t0 = 0
                for g in range(G):
                    tg = chunk_t[g]
                    gt = gpool.tile([128, 2, 128 * tg], dt.float16, tag=f"gt{g}",
                                    name=f"gt{g}")
                    nc.gpsimd.dma_gather(
                        out_ap=gt[:],
                        in_ap=t_in[:],
                        idxs_ap=ixsb[:, 8 * t0:8 * (t0 + tg)],
                        num_idxs=128 * tg,
                        num_idxs_reg=128 * tg,
                        elem_size=H,
                        transpose=True,
                        queue_num=g % 4,
                    )
                    gts.append((gt, t0, tg))
                    t0 += tg

                # Two-phase epilogue: phase 0 (all tiles except the last
                # chunk's) is emitted before the last chunk's per-tile ops, so
                # ACT/DVE chew it while waiting on the final gather; phase 1
                # covers the rest. Per-sample masked sums land in disjoint
                # npart column halves; host sums both.
                lnb = spool.tile([128, T], dt.float32, tag="lnb")
                invn = spool.tile([128, T], dt.float32, tag="invn")
                sb = spool.tile([128, T], dt.float32, tag="sb")
                eb = spool.tile([128, T], dt.float32, tag="eb")

                def epilogue(ph, pa, pb):
                    nc.scalar.activation(lnb[:, pa:pb], ssqb[:, pa:pb], A.Ln)
                    nc.scalar.activation(invn[:, pa:pb], lnb[:, pa:pb], A.Exp,
                                         scale=-0.5)
                    nc.vector.tensor_mul(sb[:, pa:pb], dotb[:, pa:pb],
                                         invn[:, pa:pb])
                    nc.scalar.activation(eb[:, pa:pb], sb[:, pa:pb], A.Exp)
                    for s in range(BS):
                        ta, tb = tile_bounds[s]
                        ta, tb = max(ta, pa), min(tb, pb)
                        if ta >= tb:
                            continue
                        col = 2 * s + (2 * BS) * ph
                        nc.vector.scalar_tensor_tensor(
                            out=eps_scr[:, ta:tb], in0=eb[:, ta:tb], scalar=0.0,
                            in1=wsb[:, ta:tb],
                            op0=OP.bypass, op1=OP.mult,
                            accum_out=npart[:, col:col + 1],
                        )
                        nc.vector.scalar_tensor_tensor(
                            out=eps_scr[:, ta:tb], in0=eb[:, ta:tb], scalar=0.0,
                            in1=wsb[:, T + ta:T + tb],
                            op0=OP.bypass, op1=OP.mult,
                            accum_out=npart[:, col + 1:col + 2],
                        )

                ph_split = T - chunk_t[-1]
                ntau = 0
                for g in range(G):
                    gt, t0, tg = gts[g]
                    for t in range(tg):
                        tau = t0 + t
                        sl = slice(128 * t, 128 * (t + 1))
                        if DO_MM:
                            P = ppool.tile([128, 128], dt.float32, tag="P",
                                           name="P")
                            nc.tensor.matmul(out=P[:], lhsT=gt[:, 0, sl],
                                             rhs=gt[:, 0, sl], start=True, stop=False)
                            nc.tensor.matmul(out=P[:], lhsT=gt[:, 1, sl],
                                             rhs=gt[:, 1, sl], start=False, stop=True)
                            if DO_DIAG:
                                nc.vector.scalar_tensor_tensor(
                                    out=dscr[ntau % N_SCR][:], in0=P[:], scalar=0.0,
                                    in1=idsb[:], op0=OP.bypass, op1=OP.mult,
                                    accum_out=ssqb[:, tau:tau + 1],
                                )
                            if DO_DOT:
                                nc.scalar.activation(
                                    dotb[:, tau:tau + 1], P[:, 127:128], A.Copy)
                        ntau += 1
                    if DO_EPI and g == G - 2 and ph_split > 0:
                        epilogue(0, 0, ph_split)
                if DO_EPI and ph_split < T:
                    epilogue(1, ph_split, T)

            nc.sync.dma_start(out=parts[:], in_=npart[:])

    nc.compile()
    return nc


def get_nc(loop_reps=0, T=None, tile_bounds=None):
    if T is None:
        T, tile_bounds = _NC_CACHE["last_layout"]
    key = (T, tuple(tile_bounds), loop_reps, DO_MM, DO_DIAG, DO_DOT, DO_EPI)
    if key not in _NC_CACHE:
        _NC_CACHE[key] = _build_nc(T, tile_bounds, loop_reps)
    return _NC_CACHE[key]


def _layout(mp, mn):
    """Per-core tile layout from the full masks. Returns (T, tile_bounds):
    tile_bounds[s] = (start, end) tile range of sample s — identical across
    cores (T_s = max over cores)."""
    tcounts = []
    for s in range(BS):
        mx = 0
        for c in range(N_CORES):
            n = int((mp[c * BS + s] | mn[c * BS + s]).sum())
            mx = max(mx, n)
        tcounts.append(max(1, -(-mx // CPT)))
    bounds = []
    t0 = 0
    for s in range(BS):
        bounds.append((t0, t0 + tcounts[s]))
        t0 += tcounts[s]
    return t0, tuple(bounds)


def _core_inputs(tm16, qhat16, mp, mn, T, tile_bounds):
    """Build one core's in_map. tm16: (ROWS, H) fp16; qhat16: (BS, H) fp16."""
    tab = np.zeros((TROWS, H), np.float16)
    tab[:ROWS] = tm16
    tab[ROWS:] = qhat16

    flat_idx = np.zeros(128 * T, np.int16)
    posw = np.zeros((128, T), np.float32)
    negw = np.zeros((128, T), np.float32)
    for s in range(BS):
        ta, tb = tile_bounds[s]
        # q slot (row 127) of every tile of this sample
        for tau in range(ta, tb):
            flat_idx[tau * 128 + 127] = ROWS + s
        m = (mp[s] | mn[s]).reshape(-1)
        cells = np.nonzero(m)[0]
        k = cells.size
        assert k <= (tb - ta) * CPT
        isp = mp[s].reshape(-1)[cells]
        kk = np.arange(k)
        tau = ta + kk // CPT
        p = kk % CPT
        flat_idx[tau * 128 + p] = (s * CELLS + cells).astype(np.int16)
        posw[p, tau] = isp.astype(np.float32)
        negw[p, tau] = (~isp).astype(np.float32)
    ix = flat_idx.reshape(8 * T, 16).T  # (16, 8T): slot g -> [g%16, g//16]
    ix = np.tile(ix, (8, 1)).copy()     # (128, 8T)
    return {
        "t_in": tab,
        "w_in": np.ascontiguousarray(np.concatenate([posw, negw], axis=1)),
        "ix_in": np.ascontiguousarray(ix),
        "id_in": np.eye(128, dtype=np.float32),
    }


def make_in_maps(pos_query, tmap, mask2d_pos, mask2d_neg):
    pq = np.asarray(pos_query, dtype=np.float32)
    tm = np.asarray(tmap, dtype=np.float32)
    mp = np.asarray(mask2d_pos).astype(bool)
    mn = np.asarray(mask2d_neg).astype(bool)

    T, tile_bounds = _layout(mp, mn)

    qn = np.sqrt(np.sum(pq * pq, axis=-1, keepdims=True, dtype=np.float32))
    qhat16 = (pq / (qn + np.float32(EPS))).astype(np.float16)
    tm16 = tm.astype(np.float16).reshape(N_CORES, ROWS, H)

    in_maps = []
    for c in range(N_CORES):
        sl = slice(c * BS, (c + 1) * BS)
        in_maps.append(_core_inputs(
            np.ascontiguousarray(tm16[c]), qhat16[sl], mp[sl], mn[sl],
            T, tile_bounds))
    return in_maps, mp, mn, T, tile_bounds


def finish(parts_per_core, mp, mn):
    """parts_per_core: list of (128, 4*BS) arrays (two phase-halves of
    per-sample partials) -> scalar loss."""
    num = np.zeros(B, np.float32)
    neg = np.zeros(B, np.float32)
    for c in range(N_CORES):
        p = parts_per_core[c]
        for s in range(BS):
            num[c * BS + s] = (p[:, 2 * s].sum(dtype=np.float32)
                               + p[:, 2 * BS + 2 * s].sum(dtype=np.float32))
            neg[c * BS + s] = (p[:, 2 * s + 1].sum(dtype=np.float32)
                               + p[:, 2 * BS + 2 * s + 1].sum(dtype=np.float32))
    den = num + neg
    with np.errstate(divide="ignore", invalid="ignore", over="ignore"):
        li = -np.log(num / (den + np.float32(EPS)))
    valid = mp.any(axis=(1, 2)) & mn.any(axis=(1, 2))
    n_valid = max(int(valid.sum()), 1)
    loss = np.where(valid, li, np.float32(0.0)).sum(dtype=np.float32) / np.float32(n_valid)
    return np.asarray(loss, dtype=np.float32)


def kernel(pos_query, tmap, mask2d_pos, mask2d_neg):
    in_maps, mp, mn, T, tile_bounds = make_in_maps(
        pos_query, tmap, mask2d_pos, mask2d_neg)
    _NC_CACHE["last_layout"] = (T, tile_bounds)
    nc = get_nc(T=T, tile_bounds=tile_bounds)
    res = run_bass_kernel_spmd(nc, in_maps, list(range(N_CORES)))
    parts_per_core = [res.results[c]["parts"] for c in range(N_CORES)]
    return finish(parts_per_core, mp, mn)


if __name__ == "__main__":
    rng = np.random.default_rng(0)
    inputs = {
        "pos_query": rng.standard_normal((B, H), dtype=np.float32),
        "tmap": rng.standard_normal((B, S, S, H), dtype=np.float32),
        "mask2d_pos": rng.random((B, S, S)) < 0.05,
        "mask2d_neg": (rng.random((B, S, S)) >= 0.05) & (rng.random((B, S, S)) < 0.35),
    }
    print(kernel(**inputs))
